# revision 50
# baseline (speedup 1.0000x reference)
"""Multi-head attention (B=2, N=2048, C=1024, H=16) on 8 trn2 NeuronCores.

Tensor-parallel over heads: core c computes heads {2c, 2c+1} for both batch
elements and emits a partial output y_c = attn_out_c @ W_out[local rows];
the host sums the 8 partials and adds b_out (+ b_v @ W_out: the V-bias
passes through softmax unchanged since the weights sum to 1, so it is
folded into the host-side bias add and never touches the device).

Per-core pipeline (single TileContext, fully unrolled):
  - Warm-up: ~24 dummy identity matmuls while the x DMAs stream, so the
    PE HAM clock-gate reaches 8/8 (2.4 GHz) before the real QKV work.
  - x^T loaded once into SBUF (bf16, host pre-transposed) as [128,1024]
    256KB transfers on the gpsimd trigger engine (8 hw DMA queues);
    weights on sync. The scalar queue carries NO DMAs: a DMA there blocks
    the first ACTIVATEs (measured a 20us pipeline stall).
  - QKV^T projection with stacked per-head weights, emitted as fine
    ~0.9us parts (4-matmul halves) deadline-scheduled into the attention
    loop so the PE never lumps >1us between exp-feeding S^T pairs.
  - S^T = K_h @ Q_h^T as TWO CONCURRENT K=64 matmuls via PE row tiling
    (tile_position (0,0)/(64,0)); measured dt=3ns between the pair.
  - P^T = exp(S^T / 32) on ScalarE straight from PSUM ([128, 1024] ops,
    ~1.11us each; ScalarE is the pacing engine). S^T/exp run LOOKAHEAD=2
    k-chunks ahead of PV so ACT never waits on the PE.
  - PV via ones-augmented V (65th stationary column = softmax denom).
    V is transposed by the DMA XBAR (dma_start_transpose, off the PE)
    and written into the vaug strip with strided copies; ones preset.
  - Normalization: pe/ss evictions (free the single-buffered PV PSUM
    early), ones-broadcast matmul, fast reciprocal, DVE multiply.
  - Output projection in fp32r (full PE rate at free-dim >= 256).

Emission order IS program order for Tile. Never emit a consumer before
its producer. Precision: bf16 operands (fp8 measured 2.3-4.9e-2 max rel
err in simulation -- over the 2e-2 budget; relative error does not
average down through sums of iid data).
"""
import sys

sys.path.insert(0, "/opt/trn_rl_repo")

import ml_dtypes
import numpy as np

import concourse.bacc as bacc
import concourse.mybir as mybir
import concourse.tile as tile
from concourse import bass_utils

F32 = mybir.dt.float32
F32R = mybir.dt.float32r
BF16 = mybir.dt.bfloat16
NPBF16 = ml_dtypes.bfloat16

EMB = 1024
HEADS = 16
B = 2
SEQ = 2048
D = 64
NCORES = 8
HPC = HEADS // NCORES          # heads per core = 2
LD = HPC * D                   # local head dim = 128
TSEQ = B * SEQ                 # 4096
CC = EMB // 128                # contraction chunks = 8
SCALE = float(EMB) ** -0.5     # 1/32

QCH = 512                      # q chunk (free dim of S^T matmuls)
NQ = SEQ // QCH                # 4 q-chunks per batch
NK = SEQ // 128                # 16 k-chunks per batch
LOOKAHEAD = 2                  # S^T/exp k-chunks in flight ahead of PV
NWARM = 34                     # HAM warm-up dummies (~3.6us busy window)


def _round_fp32r(x: np.ndarray) -> np.ndarray:
    bits = np.ascontiguousarray(x, dtype=np.float32).view(np.uint32)
    out = ((bits.astype(np.uint64) + 0x800) & 0xFFFFF000).astype(np.uint32)
    return out.view(np.float32)


def _build():
    nc = bacc.Bacc("TRN2", target_bir_lowering=False, debug=False,
                   num_devices=NCORES)

    identd = nc.dram_tensor("identd", [128, 128], BF16,
                            kind="ExternalInput")
    xT = nc.dram_tensor("xT", [CC, 128, TSEQ], BF16, kind="ExternalInput")
    wqkv = nc.dram_tensor("wqkv", [128, CC * 3 * LD], BF16,
                          kind="ExternalInput")
    bqkv = nc.dram_tensor("bqkv", [LD, 3], F32, kind="ExternalInput")
    wout = nc.dram_tensor("wout", [LD, EMB], F32R, kind="ExternalInput")
    ones = nc.dram_tensor("ones", [1, D], F32R, kind="ExternalInput")
    y = nc.dram_tensor("y", [TSEQ // 128, 128, EMB], F32,
                       kind="ExternalOutput")

    xT_c = xT.ap()
    wqkv_c = wqkv.ap()
    stq = [nc.sync, nc.gpsimd]

    with tile.TileContext(nc) as tc:
        with (
            tc.tile_pool(name="persist", bufs=1) as persist,
            tc.tile_pool(name="vt", bufs=2) as vtp,
            tc.tile_pool(name="psb", bufs=6) as psb,
            tc.tile_pool(name="norm", bufs=3) as normp,
            tc.tile_pool(name="yout", bufs=10) as youtp,
            tc.tile_pool(name="ps_st", bufs=2, space="PSUM") as ps_st,
            tc.tile_pool(name="ps_pv", bufs=1, space="PSUM") as ps_pv,
            tc.tile_pool(name="ps_misc", bufs=1, space="PSUM") as ps_misc,
            # QKV halves hold their accumulator across interleaved steps,
            # so they need a pool nothing else allocates from.
            tc.tile_pool(name="ps_qkv", bufs=1, space="PSUM") as ps_qkv,
        ):
            # ---- identity first (warm-up dummies need it ~7us in),
            # then weights, all on sync; x on gpsimd ----
            ident = persist.tile([128, 128], BF16, tag="ident")
            nc.sync.dma_start(ident[:], identd.ap())
            wall = persist.tile([128, CC * 3 * LD], BF16, tag="wall")
            wchunk = CC * 3 * LD // 3
            for i in range(3):
                nc.sync.dma_start(
                    wall[:, i * wchunk:(i + 1) * wchunk],
                    wqkv_c[:, i * wchunk:(i + 1) * wchunk])
            bqkv_sb = persist.tile([LD, 3], F32, tag="bqkv")
            nc.sync.dma_start(bqkv_sb[:], bqkv.ap())
            bias_sb = {nm: bqkv_sb[:, i:i + 1]
                       for i, nm in enumerate(("q", "k"))}

            xfull = {}

            def load_x_wpair(wp, fine=False):
                # fine: [128,512] 128KB transfers, window 0 first, split
                # gpsimd/scalar (per-queue bw ~40GB/s, so first-window
                # LATENCY needs the spread; scalar's queue is clear this
                # early and these are its only DMAs ever; NOT sync -- the
                # wall transfers hog it).  Bulk wpairs: [128,1024] gpsimd.
                ts = {}
                for kc in range(CC):
                    ts[kc] = persist.tile([128, 1024], BF16,
                                          tag=f"xf{kc}_{wp}",
                                          name=f"xf{kc}_{wp}")
                    xfull[kc, 2 * wp] = ts[kc][:, 0:512]
                    xfull[kc, 2 * wp + 1] = ts[kc][:, 512:1024]
                if fine:
                    for kc in range(CC):
                        e0 = nc.gpsimd if kc < 4 else nc.scalar
                        e0.dma_start(
                            ts[kc][:, 0:512],
                            xT_c[kc, :, wp * 1024:wp * 1024 + 512])
                    for kc in range(CC):
                        nc.scalar.dma_start(
                            ts[kc][:, 512:1024],
                            xT_c[kc, :, wp * 1024 + 512:(wp + 1) * 1024])
                else:
                    for kc in range(CC):
                        nc.gpsimd.dma_start(
                            ts[kc][:], xT_c[kc, :, wp * 1024:(wp + 1) * 1024])

            load_x_wpair(0, fine=True)

            ones_sb = persist.tile([1, D], F32R, tag="ones")
            nc.sync.dma_start(ones_sb[:], ones.ap())

            w_sb = {}
            for kc in range(CC):
                for i, nm in enumerate(("q", "k", "v")):
                    w_sb[nm, kc] = wall[:, (kc * 3 + i) * LD:
                                        (kc * 3 + i + 1) * LD]
            for wp in range(1, TSEQ // 1024):
                load_x_wpair(wp)

            # persistent activations (per batch)
            QP = [persist.tile([128, SEQ], BF16, tag=f"QP{b}",
                               name=f"QP{b}") for b in range(B)]
            KT = [persist.tile([LD, SEQ], BF16, tag=f"KT{b}", name=f"KT{b}")
                  for b in range(B)]
            outT = [persist.tile([LD, SEQ], F32R, tag=f"outT{b}",
                                 name=f"outT{b}") for b in range(B)]
            # vaug strip per batch: NK chunks x (2 heads x (64 V^T + 1 one))
            vaug = [persist.tile([128, NK, HPC, D + 1], BF16,
                                 tag=f"vaug{b}", name=f"vaug{b}")
                    for b in range(B)]
            for b in range(B):
                nc.vector.memset(vaug[b][:, :, :, D:D + 1], 1.0)

            # ---- HAM warm-up: ~5us of dense PE activity while the DMAs
            # stream, so the clock-gate is at 8/8 before the real QKV.
            # One accumulating tile => back-to-back matmuls (separate
            # tiles would WAW-serialize through semaphores and never
            # reach the HAM busy threshold).
            wps = ps_misc.tile([128, 128], F32, tag="misc")
            for i in range(NWARM):
                nc.tensor.matmul(wps[:], ident[:], ident[:],
                                 start=(i == 0), stop=(i == NWARM - 1))

            def make_parts(b, sc, k_in_misc=False):
                """QKV projection for (batch, seq-chunk) as fine-grained
                closures: k1,k2,q1,q2,v1,v2 are ~0.9us of PE each; t is
                the PE V-transpose + vaug fill.  k_in_misc pipelines the
                pre-phase K against Q (separate PSUM banks)."""
                s0 = sc * 512
                w = (b * SEQ + s0) // 512
                ps_holder = {}

                def mm_half(nm, half):
                    if half == 0:
                        pool = ps_misc if (nm == "k" and k_in_misc) \
                            else ps_qkv
                        ps_holder[nm] = pool.tile([128, 512], F32,
                                                  tag="misc" if
                                                  (nm == "k" and k_in_misc)
                                                  else "qkv",
                                                  name=f"qkv{b}{sc}{nm}")
                    ps = ps_holder[nm]
                    for kc in range(4 * half, 4 * half + 4):
                        nc.tensor.matmul(
                            ps[:], w_sb[nm, kc], xfull[kc, w],
                            start=(kc == 0), stop=(kc == CC - 1))
                    return ps

                def part_k1():
                    mm_half("k", 0)

                def part_k2():
                    ps = mm_half("k", 1)
                    nc.vector.tensor_scalar_add(
                        KT[b][:, s0:s0 + 512], ps[:], bias_sb["k"])

                def part_q1():
                    mm_half("q", 0)

                def part_q2():
                    ps = mm_half("q", 1)
                    nc.vector.tensor_scalar_add(
                        QP[b][:, s0:s0 + 512], ps[:], bias_sb["q"])

                def part_v1():
                    mm_half("v", 0)

                def part_v2():
                    ps = mm_half("v", 1)
                    vt = vtp.tile([128, 512], BF16, tag="vt")
                    # v bias handled on host (softmax-invariant)
                    nc.vector.tensor_copy(vt[:], ps[:])
                    ps_holder["vt"] = vt

                def part_t():
                    vt = ps_holder.pop("vt")
                    for j in range(4):
                        blk = sc * 4 + j
                        pst = ps_misc.tile([128, 128], BF16, tag="misc")
                        nc.tensor.transpose(
                            pst[:], vt[:, j * 128:(j + 1) * 128], ident[:])
                        nc.vector.tensor_copy(
                            vaug[b][:, blk, :, 0:D], pst[:])

                return {"k1": part_k1, "k2": part_k2, "q1": part_q1,
                        "q2": part_q2, "v1": part_v1, "v2": part_v2,
                        "t": part_t}

            parts = {(b, sc): make_parts(b, sc, k_in_misc=(b == 0 and
                                                           sc == 0))
                     for b in range(B) for sc in range(4)}

            def phase_bc(b, forced, oppo, pending=None, flush=True):
                """Attention for batch b. `forced` maps global step
                (q*NK+kc) -> list of part closures emitted at that step;
                `oppo` is a queue drained one part per otherwise-light
                step; `pending` carries deferred output projections in
                from the previous phase (returned instead of flushed when
                flush=False, so the next phase's light steps absorb them)."""
                oppo = list(oppo)
                oi = 0
                pending = list(pending or [])

                def proj_unit(b, sc, n, eng_i=None, pool=None):
                    rt = b * (SEQ // 128) + sc
                    pool = pool or ps_misc
                    ps = pool.tile([128, 512], F32,
                                   tag="misc" if pool is ps_misc else "qkv",
                                   name=f"prj{b}{sc}{n}")
                    nc.tensor.matmul(
                        ps[:], outT[b][:, sc * 128:(sc + 1) * 128],
                        wout_sb[:, n * 512:(n + 1) * 512],
                        start=True, stop=True)
                    yt = youtp.tile([128, 512], F32, tag="yt")
                    nc.vector.tensor_copy(yt[:], ps[:])
                    qi = (sc + n) if eng_i is None else eng_i
                    stq[qi % 2].dma_start(
                        y.ap()[rt, :, n * 512:(n + 1) * 512], yt[:])

                def st_exp(q, kc):
                    """S^T pair (row-tiled, concurrent) + exp -> pt."""
                    q0 = q * QCH
                    st = ps_st.tile([128, 2 * QCH], F32, tag="st")
                    k0 = kc * 128
                    for h in range(HPC):
                        nc.tensor.matmul(
                            st[:, h * QCH:(h + 1) * QCH],
                            KT[b][h * D:(h + 1) * D, k0:k0 + 128],
                            QP[b][h * D:(h + 1) * D, q0:q0 + QCH],
                            start=True, stop=True,
                            tile_position=(h * D, 0))
                    pt = psb.tile([128, 2 * QCH], BF16, tag="pt")
                    nc.scalar.activation(pt[:], st[:],
                                         mybir.ActivationFunctionType.Exp,
                                         scale=SCALE)
                    return pt

                pts = {}
                for kc in range(LOOKAHEAD):
                    pts[0, kc] = st_exp(0, kc)
                for q in range(NQ):
                    q0 = q * QCH
                    pvs = [ps_pv.tile([D + 1, QCH], F32, tag=f"pv{h}",
                                      name=f"pv{h}") for h in range(HPC)]
                    for kc in range(NK):
                        step = q * NK + kc
                        emitted = False
                        for u in forced.get(step, ()):
                            u()
                            emitted = True
                        # keep S^T/exp LOOKAHEAD chunks ahead of PV
                        nkc = kc + LOOKAHEAD
                        nq = q
                        if nkc >= NK:
                            nq, nkc = q + 1, nkc - NK
                        if nq < NQ:
                            pts[nq, nkc] = st_exp(nq, nkc)
                        pt = pts.pop((q, kc))
                        for h in range(HPC):
                            nc.tensor.matmul(
                                pvs[h][:],
                                vaug[b][:, kc, h, :],
                                pt[:, h * QCH:(h + 1) * QCH],
                                start=(kc == 0), stop=(kc == NK - 1))
                        # keep 4 projection units in reserve so the
                        # q-boundary (norm DVE chain) has independent PE
                        # work to chew on; drain harder when backlogged
                        # or in the final chunk
                        npop = 0
                        if kc in (3, 5, 7, 9):
                            npop = 1 + (len(pending) > 12)
                        elif len(pending) > 4 and kc % 2 == 0:
                            npop = 1
                        for _ in range(npop):
                            if pending:
                                pending.pop(0)()
                        if not emitted and oi < len(oppo):
                            oppo[oi]()
                            oi += 1
                    # normalize: out^T[d, q] / colsum -> outT (fp32r).
                    # pe/ss evictions free the PV PSUM bank immediately;
                    # reserved projection pops bridge the DVE latency.
                    for h in range(HPC):
                        pe = normp.tile([D, QCH], F32R, tag="pe")
                        nc.vector.tensor_copy(pe[:], pvs[h][0:D, :])
                        ss = normp.tile([1, QCH], F32R, tag="ss")
                        nc.vector.tensor_copy(ss[:], pvs[h][D:D + 1, :])
                        for _ in range(2):
                            if pending:
                                pending.pop(0)()
                        bc = ps_misc.tile([D, QCH], F32, tag="misc")
                        nc.tensor.matmul(bc[:], ones_sb[:], ss[:],
                                         start=True, stop=True)
                        rc = normp.tile([D, QCH], F32, tag="rc")
                        nc.vector.reciprocal_approx_fast(rc[:], bc[:])
                        nc.vector.tensor_mul(
                            outT[b][h * D:(h + 1) * D, q0:q0 + QCH],
                            pe[:], rc[:])
                    pending += [
                        (lambda b=b, sc=sc, n=n, eng_i=None, pool=None:
                         proj_unit(b, sc, n, eng_i, pool))
                        for sc in range(4 * q, 4 * q + 4)
                        for n in range(EMB // 512)]
                while oi < len(oppo):
                    oppo[oi]()
                    oi += 1
                if flush:
                    # final flush alternates the two 1-bank pools so the
                    # MM -> evict chains pipeline instead of serializing
                    # on a single PSUM bank (the QKV pool is free by now)
                    for j, p in enumerate(pending):
                        p(eng_i=j, pool=(ps_qkv if j % 2 else ps_misc))
                    pending = []
                return pending

            # ---- pre-phase: batch-0 seq-chunk 0 K and Q only (the seed
            # S^T/exp needs just those); V rides in at step 0 ----
            p00 = parts[0, 0]
            for nm in ("k1", "k2", "q1", "q2"):
                p00[nm]()
            wout_sb = persist.tile([LD, EMB], F32R, tag="wout")
            nc.sync.dma_start(wout_sb[:], wout.ap())

            # Batch-0's remaining QKV at hard deadlines in its q-chunk 0
            # (K(s_w) by step 4w-2 = S^T lookahead emission, V(s_w)+t by
            # step 4w = PV, q(s1) by 14, q(s2)/q(s3) early in q1/q2).
            # ALL of batch-1's k/v/t drains opportunistically through
            # batch-0's q1-q3; phase 1 keeps only its q-part deadlines.
            forced0 = {0: [p00["v1"], p00["v2"], p00["t"]]}
            for w in (1, 2, 3):
                p = parts[0, w]
                forced0[4 * w - 3] = [p["k1"]]
                forced0[4 * w - 2] = [p["k2"]]
                forced0[4 * w - 1] = [p["v1"]]
                forced0[4 * w] = forced0.get(4 * w, []) + [p["v2"], p["t"]]
            forced0[11] = forced0.get(11, []) + [parts[0, 1]["q1"]]
            forced0[13] = forced0.get(13, []) + [parts[0, 1]["q2"]]
            forced0[NK + 6] = [parts[0, 2]["q1"]]
            forced0[NK + 7] = [parts[0, 2]["q2"]]
            forced0[2 * NK + 6] = [parts[0, 3]["q1"]]
            forced0[2 * NK + 7] = [parts[0, 3]["q2"]]

            # ALL of batch-1's k/v/t (plus s0's q) drains opportunistically
            # through batch-0's attention; phase 1 keeps only its q-part
            # deadlines (light steps there feed the PE via projection pops).
            oppo0 = []
            for sc in range(4):
                p = parts[1, sc]
                oppo0 += [p["k1"], p["k2"], p["v1"], p["v2"], p["t"]]
                if sc == 0:
                    oppo0 += [p["q1"], p["q2"]]
            forced1 = {12: [parts[1, 1]["q1"]], 13: [parts[1, 1]["q2"]],
                       NK + 6: [parts[1, 2]["q1"]],
                       NK + 7: [parts[1, 2]["q2"]],
                       2 * NK + 6: [parts[1, 3]["q1"]],
                       2 * NK + 7: [parts[1, 3]["q2"]]}
            carry = phase_bc(0, forced0, oppo0, flush=False)
            phase_bc(1, forced1, [], pending=carry)

    nc.compile()
    return nc


_NC = None


def _get_nc():
    global _NC
    if _NC is None:
        _NC = _build()
    return _NC


def kernel(x, W_qkv, b_qkv, W_out, b_out):
    x = np.asarray(x, dtype=np.float32)
    W_qkv = np.asarray(W_qkv, dtype=np.float32)
    b_qkv = np.asarray(b_qkv, dtype=np.float32)
    W_out = np.asarray(W_out, dtype=np.float32)
    b_out = np.asarray(b_out, dtype=np.float32)

    nc = _get_nc()

    xT = np.ascontiguousarray(
        x.reshape(TSEQ, EMB).T.astype(NPBF16)).reshape(CC, 128, TSEQ)
    Wr = W_qkv.reshape(EMB, 3, HEADS, D)
    br = b_qkv.reshape(3, HEADS, D)
    ones = np.ones((1, D), dtype=np.float32)
    ident = np.eye(128, dtype=NPBF16)

    in_maps = []
    for c in range(NCORES):
        h0, h1 = HPC * c, HPC * (c + 1)
        in_maps.append({
            "identd": ident,
            "xT": xT,
            "wqkv": np.ascontiguousarray(
                np.stack([Wr[:, i, h0:h1].reshape(CC, 128, LD)
                          for i in range(3)], axis=1)
                .transpose(2, 0, 1, 3).reshape(128, CC * 3 * LD)
            ).astype(NPBF16),
            "bqkv": np.ascontiguousarray(
                np.stack([br[i, h0:h1].reshape(LD) for i in range(3)],
                         axis=1)),
            "wout": _round_fp32r(W_out[LD * c:LD * (c + 1)]),
            "ones": ones,
        })

    res = bass_utils.run_bass_kernel_spmd(
        nc, in_maps, core_ids=list(range(NCORES)), trace=False)

    acc = np.zeros((TSEQ // 128, 128, EMB), dtype=np.float64)
    for c in range(NCORES):
        acc += res.results[c]["y"]
    # b_out plus the softmax-invariant V-bias contribution
    bias = b_out.astype(np.float64) + b_qkv[2 * EMB:] @ W_out
    out = (acc.reshape(TSEQ, EMB) + bias).astype(np.float32)
    return out.reshape(B, SEQ, EMB)


# revision 51
# speedup vs baseline: 1.0063x; 1.0063x over previous
"""Multi-head attention (B=2, N=2048, C=1024, H=16) on 8 trn2 NeuronCores.

Tensor-parallel over heads: core c computes heads {2c, 2c+1} for both batch
elements and emits a partial output y_c = attn_out_c @ W_out[local rows];
the host sums the 8 partials and adds b_out (+ b_v @ W_out: the V-bias
passes through softmax unchanged since the weights sum to 1, so it is
folded into the host-side bias add and never touches the device).

Per-core pipeline (single TileContext, fully unrolled):
  - Warm-up: ~24 dummy identity matmuls while the x DMAs stream, so the
    PE HAM clock-gate reaches 8/8 (2.4 GHz) before the real QKV work.
  - x^T loaded once into SBUF (bf16, host pre-transposed) as [128,1024]
    256KB transfers on the gpsimd trigger engine (8 hw DMA queues);
    weights on sync. The scalar queue carries NO DMAs: a DMA there blocks
    the first ACTIVATEs (measured a 20us pipeline stall).
  - QKV^T projection with stacked per-head weights, emitted as fine
    ~0.9us parts (4-matmul halves) deadline-scheduled into the attention
    loop so the PE never lumps >1us between exp-feeding S^T pairs.
  - S^T = K_h @ Q_h^T as TWO CONCURRENT K=64 matmuls via PE row tiling
    (tile_position (0,0)/(64,0)); measured dt=3ns between the pair.
  - P^T = exp(S^T / 32) on ScalarE straight from PSUM ([128, 1024] ops,
    ~1.11us each; ScalarE is the pacing engine). S^T/exp run LOOKAHEAD=2
    k-chunks ahead of PV so ACT never waits on the PE.
  - PV via ones-augmented V (65th stationary column = softmax denom).
    V is transposed by the DMA XBAR (dma_start_transpose, off the PE)
    and written into the vaug strip with strided copies; ones preset.
  - Normalization: pe/ss evictions (free the single-buffered PV PSUM
    early), ones-broadcast matmul, fast reciprocal, DVE multiply.
  - Output projection in fp32r (full PE rate at free-dim >= 256).

Emission order IS program order for Tile. Never emit a consumer before
its producer. Precision: bf16 operands (fp8 measured 2.3-4.9e-2 max rel
err in simulation -- over the 2e-2 budget; relative error does not
average down through sums of iid data).
"""
import sys

sys.path.insert(0, "/opt/trn_rl_repo")

import ml_dtypes
import numpy as np

import concourse.bacc as bacc
import concourse.mybir as mybir
import concourse.tile as tile
from concourse import bass_utils

F32 = mybir.dt.float32
F32R = mybir.dt.float32r
BF16 = mybir.dt.bfloat16
NPBF16 = ml_dtypes.bfloat16

EMB = 1024
HEADS = 16
B = 2
SEQ = 2048
D = 64
NCORES = 8
HPC = HEADS // NCORES          # heads per core = 2
LD = HPC * D                   # local head dim = 128
TSEQ = B * SEQ                 # 4096
CC = EMB // 128                # contraction chunks = 8
SCALE = float(EMB) ** -0.5     # 1/32

QCH = 512                      # q chunk (free dim of S^T matmuls)
NQ = SEQ // QCH                # 4 q-chunks per batch
NK = SEQ // 128                # 16 k-chunks per batch
LOOKAHEAD = 2                  # S^T/exp k-chunks in flight ahead of PV
NWARM = 22                     # HAM warm-up dummies (QKV stream sustains)


def _round_fp32r(x: np.ndarray) -> np.ndarray:
    bits = np.ascontiguousarray(x, dtype=np.float32).view(np.uint32)
    out = ((bits.astype(np.uint64) + 0x800) & 0xFFFFF000).astype(np.uint32)
    return out.view(np.float32)


def _build():
    nc = bacc.Bacc("TRN2", target_bir_lowering=False, debug=False,
                   num_devices=NCORES)

    identd = nc.dram_tensor("identd", [128, 128], BF16,
                            kind="ExternalInput")
    xT = nc.dram_tensor("xT", [CC, 128, TSEQ], BF16, kind="ExternalInput")
    wqkv = nc.dram_tensor("wqkv", [128, CC * 3 * LD], BF16,
                          kind="ExternalInput")
    bqkv = nc.dram_tensor("bqkv", [LD, 3], F32, kind="ExternalInput")
    wout = nc.dram_tensor("wout", [LD, EMB], F32R, kind="ExternalInput")
    ones = nc.dram_tensor("ones", [1, D], F32R, kind="ExternalInput")
    y = nc.dram_tensor("y", [TSEQ // 128, 128, EMB], F32,
                       kind="ExternalOutput")

    xT_c = xT.ap()
    wqkv_c = wqkv.ap()
    stq = [nc.sync, nc.gpsimd]

    with tile.TileContext(nc) as tc:
        with (
            tc.tile_pool(name="persist", bufs=1) as persist,
            tc.tile_pool(name="vt", bufs=2) as vtp,
            tc.tile_pool(name="psb", bufs=8) as psb,
            tc.tile_pool(name="norm", bufs=3) as normp,
            tc.tile_pool(name="yout", bufs=12) as youtp,
            tc.tile_pool(name="ps_st", bufs=2, space="PSUM") as ps_st,
            tc.tile_pool(name="ps_pv", bufs=1, space="PSUM") as ps_pv,
            tc.tile_pool(name="ps_misc", bufs=1, space="PSUM") as ps_misc,
            # QKV halves hold their accumulator across interleaved steps,
            # so they need a pool nothing else allocates from.
            tc.tile_pool(name="ps_qkv", bufs=1, space="PSUM") as ps_qkv,
        ):
            # ---- identity first (warm-up dummies need it ~7us in),
            # then weights, all on sync; x on gpsimd ----
            ident = persist.tile([128, 128], BF16, tag="ident")
            nc.sync.dma_start(ident[:], identd.ap())
            wall = persist.tile([128, CC * 3 * LD], BF16, tag="wall")
            wchunk = CC * 3 * LD // 3
            for i in range(3):
                nc.sync.dma_start(
                    wall[:, i * wchunk:(i + 1) * wchunk],
                    wqkv_c[:, i * wchunk:(i + 1) * wchunk])
            bqkv_sb = persist.tile([LD, 3], F32, tag="bqkv")
            nc.sync.dma_start(bqkv_sb[:], bqkv.ap())
            bias_sb = {nm: bqkv_sb[:, i:i + 1]
                       for i, nm in enumerate(("q", "k"))}

            xfull = {}

            def load_x_wpair(wp, fine=False):
                # fine: [128,512] 128KB transfers, window 0 first, split
                # gpsimd/scalar (per-queue bw ~40GB/s, so first-window
                # LATENCY needs the spread; scalar's queue is clear this
                # early and these are its only DMAs ever; NOT sync -- the
                # wall transfers hog it).  Bulk wpairs: [128,1024] gpsimd.
                ts = {}
                for kc in range(CC):
                    ts[kc] = persist.tile([128, 1024], BF16,
                                          tag=f"xf{kc}_{wp}",
                                          name=f"xf{kc}_{wp}")
                    xfull[kc, 2 * wp] = ts[kc][:, 0:512]
                    xfull[kc, 2 * wp + 1] = ts[kc][:, 512:1024]
                if fine:
                    for kc in range(CC):
                        e0 = nc.gpsimd if kc < 4 else nc.scalar
                        e0.dma_start(
                            ts[kc][:, 0:512],
                            xT_c[kc, :, wp * 1024:wp * 1024 + 512])
                    for kc in range(CC):
                        nc.scalar.dma_start(
                            ts[kc][:, 512:1024],
                            xT_c[kc, :, wp * 1024 + 512:(wp + 1) * 1024])
                else:
                    for kc in range(CC):
                        nc.gpsimd.dma_start(
                            ts[kc][:], xT_c[kc, :, wp * 1024:(wp + 1) * 1024])

            load_x_wpair(0, fine=True)

            ones_sb = persist.tile([1, D], F32R, tag="ones")
            nc.sync.dma_start(ones_sb[:], ones.ap())

            w_sb = {}
            for kc in range(CC):
                for i, nm in enumerate(("q", "k", "v")):
                    w_sb[nm, kc] = wall[:, (kc * 3 + i) * LD:
                                        (kc * 3 + i + 1) * LD]
            for wp in range(1, TSEQ // 1024):
                load_x_wpair(wp)

            # persistent activations (per batch)
            QP = [persist.tile([128, SEQ], BF16, tag=f"QP{b}",
                               name=f"QP{b}") for b in range(B)]
            KT = [persist.tile([LD, SEQ], BF16, tag=f"KT{b}", name=f"KT{b}")
                  for b in range(B)]
            outT = [persist.tile([LD, SEQ], F32R, tag=f"outT{b}",
                                 name=f"outT{b}") for b in range(B)]
            # vaug strip per batch: NK chunks x (2 heads x (64 V^T + 1 one))
            vaug = [persist.tile([128, NK, HPC, D + 1], BF16,
                                 tag=f"vaug{b}", name=f"vaug{b}")
                    for b in range(B)]
            for b in range(B):
                nc.vector.memset(vaug[b][:, :, :, D:D + 1], 1.0)

            # ---- HAM warm-up: ~5us of dense PE activity while the DMAs
            # stream, so the clock-gate is at 8/8 before the real QKV.
            # One accumulating tile => back-to-back matmuls (separate
            # tiles would WAW-serialize through semaphores and never
            # reach the HAM busy threshold).
            wps = ps_misc.tile([128, 128], F32, tag="misc")
            for i in range(NWARM):
                nc.tensor.matmul(wps[:], ident[:], ident[:],
                                 start=(i == 0), stop=(i == NWARM - 1))

            def make_parts(b, sc, k_in_misc=False):
                """QKV projection for (batch, seq-chunk) as fine-grained
                closures: k1,k2,q1,q2,v1,v2 are ~0.9us of PE each; t is
                the PE V-transpose + vaug fill.  k_in_misc pipelines the
                pre-phase K against Q (separate PSUM banks)."""
                s0 = sc * 512
                w = (b * SEQ + s0) // 512
                ps_holder = {}

                def mm_half(nm, half):
                    if half == 0:
                        pool = ps_misc if (nm == "k" and k_in_misc) \
                            else ps_qkv
                        ps_holder[nm] = pool.tile([128, 512], F32,
                                                  tag="misc" if
                                                  (nm == "k" and k_in_misc)
                                                  else "qkv",
                                                  name=f"qkv{b}{sc}{nm}")
                    ps = ps_holder[nm]
                    for kc in range(4 * half, 4 * half + 4):
                        nc.tensor.matmul(
                            ps[:], w_sb[nm, kc], xfull[kc, w],
                            start=(kc == 0), stop=(kc == CC - 1))
                    return ps

                def part_k1():
                    mm_half("k", 0)

                def part_k2():
                    ps = mm_half("k", 1)
                    nc.vector.tensor_scalar_add(
                        KT[b][:, s0:s0 + 512], ps[:], bias_sb["k"])

                def part_q1():
                    mm_half("q", 0)

                def part_q2():
                    ps = mm_half("q", 1)
                    nc.vector.tensor_scalar_add(
                        QP[b][:, s0:s0 + 512], ps[:], bias_sb["q"])

                def part_v1():
                    mm_half("v", 0)

                def part_v2():
                    ps = mm_half("v", 1)
                    vt = vtp.tile([128, 512], BF16, tag="vt")
                    # v bias handled on host (softmax-invariant)
                    nc.vector.tensor_copy(vt[:], ps[:])
                    ps_holder["vt"] = vt

                def part_t():
                    vt = ps_holder.pop("vt")
                    for j in range(4):
                        blk = sc * 4 + j
                        pst = ps_misc.tile([128, 128], BF16, tag="misc")
                        nc.tensor.transpose(
                            pst[:], vt[:, j * 128:(j + 1) * 128], ident[:])
                        nc.vector.tensor_copy(
                            vaug[b][:, blk, :, 0:D], pst[:])

                return {"k1": part_k1, "k2": part_k2, "q1": part_q1,
                        "q2": part_q2, "v1": part_v1, "v2": part_v2,
                        "t": part_t}

            parts = {(b, sc): make_parts(b, sc, k_in_misc=(b == 0 and
                                                           sc == 0))
                     for b in range(B) for sc in range(4)}

            def phase_bc(b, forced, oppo, pending=None, flush=True):
                """Attention for batch b. `forced` maps global step
                (q*NK+kc) -> list of part closures emitted at that step;
                `oppo` is a queue drained one part per otherwise-light
                step; `pending` carries deferred output projections in
                from the previous phase (returned instead of flushed when
                flush=False, so the next phase's light steps absorb them)."""
                oppo = list(oppo)
                oi = 0
                pending = list(pending or [])

                def proj_unit(b, sc, n, eng_i=None, pool=None):
                    rt = b * (SEQ // 128) + sc
                    pool = pool or ps_misc
                    ps = pool.tile([128, 512], F32,
                                   tag="misc" if pool is ps_misc else "qkv",
                                   name=f"prj{b}{sc}{n}")
                    nc.tensor.matmul(
                        ps[:], outT[b][:, sc * 128:(sc + 1) * 128],
                        wout_sb[:, n * 512:(n + 1) * 512],
                        start=True, stop=True)
                    yt = youtp.tile([128, 512], F32, tag="yt")
                    nc.vector.tensor_copy(yt[:], ps[:])
                    qi = (sc + n) if eng_i is None else eng_i
                    stq[qi % 2].dma_start(
                        y.ap()[rt, :, n * 512:(n + 1) * 512], yt[:])

                def st_exp(q, kc):
                    """S^T pair (row-tiled, concurrent) + exp -> pt."""
                    q0 = q * QCH
                    st = ps_st.tile([128, 2 * QCH], F32, tag="st")
                    k0 = kc * 128
                    for h in range(HPC):
                        nc.tensor.matmul(
                            st[:, h * QCH:(h + 1) * QCH],
                            KT[b][h * D:(h + 1) * D, k0:k0 + 128],
                            QP[b][h * D:(h + 1) * D, q0:q0 + QCH],
                            start=True, stop=True,
                            tile_position=(h * D, 0))
                    pt = psb.tile([128, 2 * QCH], BF16, tag="pt")
                    nc.scalar.activation(pt[:], st[:],
                                         mybir.ActivationFunctionType.Exp,
                                         scale=SCALE)
                    return pt

                pts = {}
                for kc in range(LOOKAHEAD):
                    pts[0, kc] = st_exp(0, kc)
                for q in range(NQ):
                    q0 = q * QCH
                    pvs = [ps_pv.tile([D + 1, QCH], F32, tag=f"pv{h}",
                                      name=f"pv{h}") for h in range(HPC)]
                    for kc in range(NK):
                        step = q * NK + kc
                        emitted = False
                        for u in forced.get(step, ()):
                            u()
                            emitted = True
                        # keep S^T/exp LOOKAHEAD chunks ahead of PV
                        nkc = kc + LOOKAHEAD
                        nq = q
                        if nkc >= NK:
                            nq, nkc = q + 1, nkc - NK
                        if nq < NQ:
                            pts[nq, nkc] = st_exp(nq, nkc)
                        pt = pts.pop((q, kc))
                        for h in range(HPC):
                            nc.tensor.matmul(
                                pvs[h][:],
                                vaug[b][:, kc, h, :],
                                pt[:, h * QCH:(h + 1) * QCH],
                                start=(kc == 0), stop=(kc == NK - 1))
                        # keep 4 projection units in reserve so the
                        # q-boundary (norm DVE chain) has independent PE
                        # work to chew on; drain harder when backlogged
                        # or in the final chunk
                        npop = 0
                        if kc in (3, 5, 7, 9):
                            npop = 1 + (len(pending) > 12)
                        elif len(pending) > 4 and kc % 2 == 0:
                            npop = 1
                        for _ in range(npop):
                            if pending:
                                pending.pop(0)()
                        if not emitted and oi < len(oppo):
                            oppo[oi]()
                            oi += 1
                    # normalize: out^T[d, q] / colsum -> outT (fp32r).
                    # pe/ss evictions free the PV PSUM bank immediately;
                    # reserved projection pops bridge the DVE latency.
                    for h in range(HPC):
                        pe = normp.tile([D, QCH], F32R, tag="pe")
                        nc.vector.tensor_copy(pe[:], pvs[h][0:D, :])
                        ss = normp.tile([1, QCH], F32R, tag="ss")
                        nc.vector.tensor_copy(ss[:], pvs[h][D:D + 1, :])
                        for _ in range(2):
                            if pending:
                                pending.pop(0)()
                        bc = ps_misc.tile([D, QCH], F32, tag="misc")
                        nc.tensor.matmul(bc[:], ones_sb[:], ss[:],
                                         start=True, stop=True)
                        rc = normp.tile([D, QCH], F32, tag="rc")
                        nc.vector.reciprocal_approx_fast(rc[:], bc[:])
                        nc.vector.tensor_mul(
                            outT[b][h * D:(h + 1) * D, q0:q0 + QCH],
                            pe[:], rc[:])
                    pending += [
                        (lambda b=b, sc=sc, n=n, eng_i=None, pool=None:
                         proj_unit(b, sc, n, eng_i, pool))
                        for sc in range(4 * q, 4 * q + 4)
                        for n in range(EMB // 512)]
                while oi < len(oppo):
                    oppo[oi]()
                    oi += 1
                if flush:
                    # final flush alternates the two 1-bank pools so the
                    # MM -> evict chains pipeline instead of serializing
                    # on a single PSUM bank (the QKV pool is free by now)
                    for j, p in enumerate(pending):
                        p(eng_i=j, pool=(ps_qkv if j % 2 else ps_misc))
                    pending = []
                return pending

            # ---- pre-phase: batch-0 seq-chunk 0 K and Q only (the seed
            # S^T/exp needs just those); V rides in at step 0 ----
            p00 = parts[0, 0]
            for nm in ("k1", "k2", "q1", "q2"):
                p00[nm]()
            wout_sb = persist.tile([LD, EMB], F32R, tag="wout")
            nc.sync.dma_start(wout_sb[:], wout.ap())

            # Batch-0's remaining QKV at hard deadlines in its q-chunk 0
            # (K(s_w) by step 4w-2 = S^T lookahead emission, V(s_w)+t by
            # step 4w = PV, q(s1) by 14, q(s2)/q(s3) early in q1/q2).
            # ALL of batch-1's k/v/t drains opportunistically through
            # batch-0's q1-q3; phase 1 keeps only its q-part deadlines.
            forced0 = {0: [p00["v1"], p00["v2"], p00["t"]]}
            for w in (1, 2, 3):
                p = parts[0, w]
                forced0[4 * w - 3] = [p["k1"]]
                forced0[4 * w - 2] = [p["k2"]]
                forced0[4 * w - 1] = [p["v1"]]
                forced0[4 * w] = forced0.get(4 * w, []) + [p["v2"], p["t"]]
            forced0[11] = forced0.get(11, []) + [parts[0, 1]["q1"]]
            forced0[13] = forced0.get(13, []) + [parts[0, 1]["q2"]]
            forced0[NK + 6] = [parts[0, 2]["q1"]]
            forced0[NK + 7] = [parts[0, 2]["q2"]]
            forced0[2 * NK + 6] = [parts[0, 3]["q1"]]
            forced0[2 * NK + 7] = [parts[0, 3]["q2"]]

            # ALL of batch-1's k/v/t (plus s0's q) drains opportunistically
            # through batch-0's attention; phase 1 keeps only its q-part
            # deadlines (light steps there feed the PE via projection pops).
            oppo0 = []
            for sc in range(4):
                p = parts[1, sc]
                oppo0 += [p["k1"], p["k2"], p["v1"], p["v2"], p["t"]]
                if sc == 0:
                    oppo0 += [p["q1"], p["q2"]]
            forced1 = {12: [parts[1, 1]["q1"]], 13: [parts[1, 1]["q2"]],
                       NK + 6: [parts[1, 2]["q1"]],
                       NK + 7: [parts[1, 2]["q2"]],
                       2 * NK + 6: [parts[1, 3]["q1"]],
                       2 * NK + 7: [parts[1, 3]["q2"]]}
            carry = phase_bc(0, forced0, oppo0, flush=False)
            phase_bc(1, forced1, [], pending=carry)

    nc.compile()
    return nc


_NC = None


def _get_nc():
    global _NC
    if _NC is None:
        _NC = _build()
    return _NC


def kernel(x, W_qkv, b_qkv, W_out, b_out):
    x = np.asarray(x, dtype=np.float32)
    W_qkv = np.asarray(W_qkv, dtype=np.float32)
    b_qkv = np.asarray(b_qkv, dtype=np.float32)
    W_out = np.asarray(W_out, dtype=np.float32)
    b_out = np.asarray(b_out, dtype=np.float32)

    nc = _get_nc()

    xT = np.ascontiguousarray(
        x.reshape(TSEQ, EMB).T.astype(NPBF16)).reshape(CC, 128, TSEQ)
    Wr = W_qkv.reshape(EMB, 3, HEADS, D)
    br = b_qkv.reshape(3, HEADS, D)
    ones = np.ones((1, D), dtype=np.float32)
    ident = np.eye(128, dtype=NPBF16)

    in_maps = []
    for c in range(NCORES):
        h0, h1 = HPC * c, HPC * (c + 1)
        in_maps.append({
            "identd": ident,
            "xT": xT,
            "wqkv": np.ascontiguousarray(
                np.stack([Wr[:, i, h0:h1].reshape(CC, 128, LD)
                          for i in range(3)], axis=1)
                .transpose(2, 0, 1, 3).reshape(128, CC * 3 * LD)
            ).astype(NPBF16),
            "bqkv": np.ascontiguousarray(
                np.stack([br[i, h0:h1].reshape(LD) for i in range(3)],
                         axis=1)),
            "wout": _round_fp32r(W_out[LD * c:LD * (c + 1)]),
            "ones": ones,
        })

    res = bass_utils.run_bass_kernel_spmd(
        nc, in_maps, core_ids=list(range(NCORES)), trace=False)

    acc = np.zeros((TSEQ // 128, 128, EMB), dtype=np.float64)
    for c in range(NCORES):
        acc += res.results[c]["y"]
    # b_out plus the softmax-invariant V-bias contribution
    bias = b_out.astype(np.float64) + b_qkv[2 * EMB:] @ W_out
    out = (acc.reshape(TSEQ, EMB) + bias).astype(np.float32)
    return out.reshape(B, SEQ, EMB)
